# revision 1
# baseline (speedup 1.0000x reference)
"""Trainium2 Bass kernel: PhaseMultiHeadModel (complex phase attention + complex FF
+ ComplexNorm + vocab readout), SPMD over 8 NeuronCores.

Strategy: replicate the (cheap) state/attention/FF/norm pipeline on every core,
column-shard the vocab readout matrices (w_r/w_i) 8 ways, no collectives.
Each core outputs a [Vchunk, tokens] logits slab; host concatenates over vocab.

All heavy math runs on device in f32 / f32r (fp22 matmuls at full PE rate).
Host-side work is limited to input sharding / constant table prep (cos/sin of
rotation angles and positional phases, causal step mask, weight re-layouts).
"""

import math

import numpy as np

P = 128
NCORES = 8
SBLK = 512  # attention s-block / moving free dim


def build_nc(B, S, V, D, H, VCP):
    """Build the single-core Bass program (identical on all cores)."""
    import concourse.bass as bass
    import concourse.mybir as mybir
    import concourse.tile as tile
    from concourse import bacc
    from concourse.masks import make_identity

    HD = D // H
    SB = B * S
    NT = SB // P  # token tiles
    DT = D // P   # 128-row blocks of d
    KT = 2 * DT   # readout contraction tiles (re + im)
    NVT = VCP // P
    assert HD == 64 and D == H * HD and S % SBLK == 0 and SB % P == 0

    f32 = mybir.dt.float32
    f32r = mybir.dt.float32r
    i32 = mybir.dt.int32
    AF = mybir.ActivationFunctionType
    OP = mybir.AluOpType

    def r(ap):
        return ap.bitcast(f32r)

    nc = bacc.Bacc()

    xidx = nc.dram_tensor("xidx", [SB, 1], i32, kind="ExternalInput")
    embt = nc.dram_tensor("emb", [V, D], f32, kind="ExternalInput")
    cph = nc.dram_tensor("cph", [SB, D], f32, kind="ExternalInput")
    sph = nc.dram_tensor("sph", [SB, D], f32, kind="ExternalInput")
    mkt = nc.dram_tensor("mk", [H, P, P], f32, kind="ExternalInput")
    mvt = nc.dram_tensor("mv", [H, P, P], f32, kind="ExternalInput")
    stepm = nc.dram_tensor("stepm", [P, 1280], f32, kind="ExternalInput")
    onesd = nc.dram_tensor("ones", [P, P], f32, kind="ExternalInput")
    ffA = nc.dram_tensor("ffA", [H, P, D], f32, kind="ExternalInput")
    ffB = nc.dram_tensor("ffB", [H, P, D], f32, kind="ExternalInput")
    w2t = nc.dram_tensor("w2t", [KT, P, VCP], f32, kind="ExternalInput")
    bias2 = nc.dram_tensor("bias2", [P, NVT], f32, kind="ExternalInput")
    outv = nc.dram_tensor("outv", [VCP, SB], f32, kind="ExternalOutput")

    EPS = 1.0e-5

    ctx_lp = nc.allow_low_precision(reason="fp32r (fp22) compute is intentional")
    ctx_lp.__enter__()
    with tile.TileContext(nc) as tc:
        with (
            tc.tile_pool(name="const", bufs=1) as cpool,
            tc.tile_pool(name="zfm", bufs=1) as zpool,
        ):
            ident = cpool.tile([P, P], f32)
            make_identity(nc, ident[:])
            ones_col = cpool.tile([P, 1], f32r)
            nc.sync.dma_start(ones_col[:], r(onesd[:, 0:1]))
            ones_row = cpool.tile([1, P], f32r)
            nc.sync.dma_start(ones_row[:], r(onesd[0:1, :]))
            stepm_sb = cpool.tile([P, 1280], f32)
            nc.sync.dma_start(stepm_sb[:], stepm[:])
            bias_sb = cpool.tile([P, NVT], f32)
            nc.sync.dma_start(bias_sb[:], bias2[:])

            # zFM pre-FF: block h = [sr rows of head h (64) ; si rows (64)],
            # feature-major over tokens. Post-FF: blocks 0..DT-1 = re j-tiles,
            # blocks DT..2DT-1 = im j-tiles.
            zFM = zpool.tile([P, H, SB], f32r)

            # ---------------- Phase 1: embed + phase -> zFM ----------------
            with (
                tc.tile_pool(name="p1", bufs=2) as p1,
                tc.tile_pool(name="p1ps", bufs=4, space="PSUM") as p1ps,
            ):
                for ti in range(NT):
                    t0 = ti * P
                    idx = p1.tile([P, 1], i32, tag="idx")
                    nc.sync.dma_start(idx[:], xidx[t0 : t0 + P, :])
                    g = p1.tile([P, D], f32, tag="g")
                    nc.gpsimd.indirect_dma_start(
                        out=g[:],
                        out_offset=None,
                        in_=embt[:],
                        in_offset=bass.IndirectOffsetOnAxis(ap=idx[:, :1], axis=0),
                    )
                    mag = p1.tile([P, D], f32, tag="mag")
                    nc.scalar.activation(mag[:], g[:], AF.Tanh)
                    cpt = p1.tile([P, D], f32, tag="cpt")
                    nc.sync.dma_start(cpt[:], cph[t0 : t0 + P, :])
                    spt = p1.tile([P, D], f32, tag="spt")
                    nc.sync.dma_start(spt[:], sph[t0 : t0 + P, :])
                    # token-major state with sr/si interleaved per head in the
                    # free dim, so one 128x128 transpose yields the [sr_h; si_h]
                    # feature-major block directly (psum base partition 0).
                    zt = p1.tile([P, H, P], f32, tag="zt")
                    for h in range(H):
                        hs = slice(h * HD, (h + 1) * HD)
                        nc.vector.tensor_mul(
                            zt[:, h, 0:HD], mag[:, hs], cpt[:, hs]
                        )
                        nc.vector.tensor_mul(
                            zt[:, h, HD:P], mag[:, hs], spt[:, hs]
                        )
                    for h in range(H):
                        ps = p1ps.tile([P, P], f32, tag="tp")
                        nc.tensor.transpose(ps[:], zt[:, h, :], ident[:])
                        if h % 2 == 0:
                            nc.scalar.copy(zFM[:, h, t0 : t0 + P], ps[:])
                        else:
                            nc.vector.tensor_copy(zFM[:, h, t0 : t0 + P], ps[:])

            # ---------------- Phase 2: attention per head ----------------
            with (
                tc.tile_pool(name="hd", bufs=1) as hp,
                tc.tile_pool(name="rot", bufs=2) as rp_,
                tc.tile_pool(name="exp", bufs=3) as ep,
                tc.tile_pool(name="sm", bufs=2) as smp,
                tc.tile_pool(name="stps", bufs=2, space="PSUM") as stps,
                tc.tile_pool(name="pvps", bufs=1, space="PSUM") as pvps,
                tc.tile_pool(name="smps", bufs=1, space="PSUM") as smps,
                tc.tile_pool(name="rpps", bufs=1, space="PSUM") as rpps,
                tc.tile_pool(name="cbps", bufs=3, space="PSUM") as cbps,
            ):
                for h in range(H):
                    # k2 / v2 via per-head 128x128 rotation matrices on PE
                    mk_sb = rp_.tile([P, P], f32r, tag="mk")
                    nc.sync.dma_start(mk_sb[:], r(mkt[h, :, :]))
                    mv_sb = rp_.tile([P, P], f32r, tag="mv")
                    nc.sync.dma_start(mv_sb[:], r(mvt[h, :, :]))
                    k2h = hp.tile([P, SB], f32r, tag="k2h")
                    v2fm = hp.tile([P, SB], f32, tag="v2fm")
                    for tg in range(SB // SBLK):
                        sl = slice(tg * SBLK, (tg + 1) * SBLK)
                        kps = cbps.tile([P, SBLK], f32, tag="cb")
                        nc.tensor.matmul(
                            kps[:], lhsT=r(mk_sb[:]), rhs=r(zFM[:, h, sl]),
                            start=True, stop=True,
                        )
                        nc.scalar.copy(k2h[:, sl], kps[:])
                        vps = cbps.tile([P, SBLK], f32, tag="cb")
                        nc.tensor.matmul(
                            vps[:], lhsT=r(mv_sb[:]), rhs=r(zFM[:, h, sl]),
                            start=True, stop=True,
                        )
                        nc.vector.tensor_copy(v2fm[:, sl], vps[:])
                    # token-major v2h via PE transpose
                    v2h = hp.tile([P, SB // P, P], f32r, tag="v2h")
                    for tb in range(SB // P):
                        ps = cbps.tile([P, P], f32, tag="cb")
                        nc.tensor.transpose(
                            ps[:], v2fm[:, tb * P : (tb + 1) * P], ident[:]
                        )
                        if tb % 2 == 0:
                            nc.scalar.copy(v2h[:, tb, :], ps[:])
                        else:
                            nc.vector.tensor_copy(v2h[:, tb, :], ps[:])

                    for b in range(B):
                        q2 = zFM[:, h, b * S : (b + 1) * S]
                        for sbi in range(S // SBLK):
                            s0 = sbi * SBLK
                            ntt = (s0 + SBLK) // P
                            pv = pvps.tile([P, SBLK], f32, tag="pv")
                            sm = smps.tile([1, SBLK], f32, tag="sm")
                            for tt in range(ntt):
                                t0 = tt * P
                                st = stps.tile([P, SBLK], f32, tag="st")
                                nc.tensor.matmul(
                                    st[:],
                                    lhsT=r(k2h[:, b * S + t0 : b * S + t0 + P]),
                                    rhs=r(q2[:, s0 : s0 + SBLK]),
                                    start=True,
                                    stop=True,
                                )
                                e = ep.tile([P, SBLK], f32r, tag="e")
                                nc.scalar.activation(e[:], st[:], AF.Exp)
                                if t0 + P - 1 > s0:
                                    off = 640 + (s0 - t0)
                                    nc.vector.tensor_mul(
                                        e[:], e[:], stepm_sb[:, off : off + SBLK]
                                    )
                                nc.tensor.matmul(
                                    pv[:],
                                    lhsT=r(v2h[:, b * (S // P) + tt, :]),
                                    rhs=r(e[:]),
                                    start=(tt == 0),
                                    stop=(tt == ntt - 1),
                                )
                                nc.tensor.matmul(
                                    sm[:],
                                    lhsT=r(ones_col[:]),
                                    rhs=r(e[:]),
                                    start=(tt == 0),
                                    stop=(tt == ntt - 1),
                                )
                            rc = smp.tile([1, SBLK], f32r, tag="rc")
                            nc.vector.reciprocal(rc[:], sm[:])
                            rp = rpps.tile([P, SBLK], f32, tag="rp")
                            nc.tensor.matmul(
                                rp[:],
                                lhsT=r(ones_row[:]),
                                rhs=r(rc[:]),
                                start=True,
                                stop=True,
                            )
                            rps = smp.tile([P, SBLK], f32, tag="rps")
                            nc.scalar.copy(rps[:], rp[:])
                            tmp = smp.tile([P, SBLK], f32, tag="tmp")
                            nc.vector.tensor_mul(tmp[:], pv[:], rps[:])
                            dst = slice(b * S + s0, b * S + s0 + SBLK)
                            nc.vector.tensor_add(
                                zFM[0:HD, h, dst], zFM[0:HD, h, dst], tmp[0:HD, :]
                            )
                            nc.vector.tensor_add(
                                zFM[HD:P, h, dst], zFM[HD:P, h, dst], tmp[HD:P, :]
                            )

            # ---------------- Phase 3: FF + ComplexNorm (per token slice) ----
            TSL = 256
            with (
                tc.tile_pool(name="ffw", bufs=3) as fwp,
                tc.tile_pool(name="fo", bufs=1) as fop,
                tc.tile_pool(name="nrm", bufs=2) as nmp,
                tc.tile_pool(name="ffps", bufs=2, space="PSUM") as ffps,
                tc.tile_pool(name="stat", bufs=1, space="PSUM") as stat,
                tc.tile_pool(name="nrps", bufs=1, space="PSUM") as nrps,
            ):
                for tti in range(SB // TSL):
                    ts0 = tti * TSL
                    tsl = slice(ts0, ts0 + TSL)
                    fre = fop.tile([P, DT, TSL], f32, tag="fre")
                    fim = fop.tile([P, DT, TSL], f32, tag="fim")
                    fm = fop.tile([P, DT, TSL], f32r, tag="fm")
                    ssum = stat.tile([1, TSL], f32, tag="ssum")
                    ssq = stat.tile([1, TSL], f32, tag="ssq")
                    for jt in range(DT):
                        pre = ffps.tile([P, TSL], f32, tag="pre")
                        pim = ffps.tile([P, TSL], f32, tag="pim")
                        for kt in range(H):
                            fa = fwp.tile([P, P], f32r, tag="fa")
                            nc.sync.dma_start(
                                fa[:], r(ffA[kt, :, jt * P : (jt + 1) * P])
                            )
                            fb = fwp.tile([P, P], f32r, tag="fb")
                            nc.sync.dma_start(
                                fb[:], r(ffB[kt, :, jt * P : (jt + 1) * P])
                            )
                            rhs = r(zFM[:, kt, tsl])
                            nc.tensor.matmul(
                                pre[:], lhsT=r(fa[:]), rhs=rhs,
                                start=(kt == 0), stop=(kt == H - 1),
                            )
                            nc.tensor.matmul(
                                pim[:], lhsT=r(fb[:]), rhs=rhs,
                                start=(kt == 0), stop=(kt == H - 1),
                            )
                        nc.scalar.copy(fre[:, jt, :], pre[:])
                        nc.scalar.copy(fim[:, jt, :], pim[:])
                        sq = nmp.tile([P, TSL], f32r, tag="sq")
                        nc.vector.tensor_mul(sq[:], fre[:, jt, :], fre[:, jt, :])
                        sq2 = nmp.tile([P, TSL], f32, tag="sq2")
                        nc.vector.tensor_mul(sq2[:], fim[:, jt, :], fim[:, jt, :])
                        nc.vector.tensor_add(sq[:], sq[:], sq2[:])
                        nc.scalar.activation(fm[:, jt, :], sq[:], AF.Sqrt)
                        nc.tensor.matmul(
                            ssum[:], lhsT=r(ones_col[:]), rhs=r(fm[:, jt, :]),
                            start=(jt == 0), stop=(jt == DT - 1),
                        )
                        nc.tensor.matmul(
                            ssq[:], lhsT=r(ones_col[:]), rhs=r(sq[:]),
                            start=(jt == 0), stop=(jt == DT - 1),
                        )
                    # stats: mean = ssum/D ; var = (ssq - mean*ssum)/(D-1)
                    mean = nmp.tile([1, TSL], f32r, tag="mean")
                    nc.vector.tensor_scalar_mul(mean[:], ssum[:], 1.0 / D)
                    q1 = nmp.tile([1, TSL], f32, tag="q1")
                    nc.vector.tensor_mul(q1[:], mean[:], ssum[:])
                    var = nmp.tile([1, TSL], f32, tag="var")
                    nc.vector.tensor_sub(var[:], ssq[:], q1[:])
                    nc.vector.tensor_scalar_mul(var[:], var[:], 1.0 / (D - 1))
                    std = nmp.tile([1, TSL], f32, tag="std")
                    nc.scalar.activation(std[:], var[:], AF.Sqrt)
                    nc.vector.tensor_scalar_add(std[:], std[:], EPS)
                    rstd = nmp.tile([1, TSL], f32r, tag="rstd")
                    nc.vector.reciprocal(rstd[:], std[:])
                    # replicate mean/rstd across partitions via ones matmul
                    mrp = nrps.tile([P, TSL], f32, tag="mrp")
                    nc.tensor.matmul(
                        mrp[:], lhsT=r(ones_row[:]), rhs=r(mean[:]),
                        start=True, stop=True,
                    )
                    mrep = nmp.tile([P, TSL], f32, tag="mrep")
                    nc.scalar.copy(mrep[:], mrp[:])
                    rrp = nrps.tile([P, TSL], f32, tag="rrp")
                    nc.tensor.matmul(
                        rrp[:], lhsT=r(ones_row[:]), rhs=r(rstd[:]),
                        start=True, stop=True,
                    )
                    rrep = nmp.tile([P, TSL], f32, tag="rrep")
                    nc.scalar.copy(rrep[:], rrp[:])
                    for jt in range(DT):
                        xm = nmp.tile([P, TSL], f32, tag="xm")
                        nc.vector.tensor_sub(xm[:], fm[:, jt, :], mrep[:])
                        nc.vector.tensor_mul(xm[:], xm[:], rrep[:])
                        th = nmp.tile([P, TSL], f32, tag="th")
                        nc.scalar.activation(th[:], xm[:], AF.Tanh)
                        rm = nmp.tile([P, TSL], f32, tag="rm")
                        nc.vector.tensor_scalar_add(rm[:], fm[:, jt, :], EPS)
                        nc.vector.reciprocal(rm[:], rm[:])
                        nc.vector.tensor_mul(th[:], th[:], rm[:])
                        nc.vector.tensor_mul(zFM[:, jt, tsl], fre[:, jt, :], th[:])
                        nc.vector.tensor_mul(
                            zFM[:, DT + jt, tsl], fim[:, jt, :], th[:]
                        )

            # ---------------- Phase 4: vocab readout ----------------
            with (
                tc.tile_pool(name="w2", bufs=4) as wp,
                tc.tile_pool(name="ob", bufs=2) as op_,
                tc.tile_pool(name="rops", bufs=2, space="PSUM") as rops,
            ):
                for vt in range(NVT):
                    ps = rops.tile([P, SB], f32, tag="ro")
                    for kt in range(KT):
                        w = wp.tile([P, P], f32r, tag="w")
                        nc.sync.dma_start(w[:], r(w2t[kt, :, vt * P : (vt + 1) * P]))
                        for tg in range(SB // SBLK):
                            nc.tensor.matmul(
                                ps[:, tg * SBLK : (tg + 1) * SBLK],
                                lhsT=r(w[:]),
                                rhs=r(zFM[:, kt, tg * SBLK : (tg + 1) * SBLK]),
                                start=(kt == 0),
                                stop=(kt == KT - 1),
                            )
                    ob = op_.tile([P, SB], f32, tag="ob")
                    nc.vector.tensor_scalar_add(
                        ob[:], ps[:], bias_sb[:, vt : vt + 1]
                    )
                    nc.sync.dma_start(outv[vt * P : (vt + 1) * P, :], ob[:])

    ctx_lp.__exit__(None, None, None)
    nc.compile()
    return nc


def host_prep(x, emb, q_rot, k_rot, v_rot, ff_real, ff_imag, w_r, b_r, w_i, b_i,
              ncores=NCORES, vcp=None):
    """Host-side sharding + constant table prep. Returns (common, per_core)."""
    x = np.asarray(x)
    emb = np.asarray(emb, np.float32)
    q_rot = np.asarray(q_rot, np.float32)
    k_rot = np.asarray(k_rot, np.float32)
    v_rot = np.asarray(v_rot, np.float32)
    ff_real = np.asarray(ff_real, np.float32)
    ff_imag = np.asarray(ff_imag, np.float32)
    w_r = np.asarray(w_r, np.float32)
    w_i = np.asarray(w_i, np.float32)
    b_r = np.asarray(b_r, np.float32)
    b_i = np.asarray(b_i, np.float32)

    B, S = x.shape
    V, D = emb.shape
    H, HD = q_rot.shape
    SB = B * S
    DT = D // P
    KT = 2 * DT
    Vc = V // ncores
    if vcp is None:
        vcp = ((Vc + P - 1) // P) * P

    pos = np.arange(S, dtype=np.float32)[:, None]
    dim = np.arange(D, dtype=np.float32)[None, :]
    freq = np.exp(-(dim / D) * np.float32(math.log(10000.0)))
    ph = pos * freq * np.float32(math.pi)
    cph = np.tile(np.cos(ph), (B, 1)).astype(np.float32)
    sph = np.tile(np.sin(ph), (B, 1)).astype(np.float32)

    delta = q_rot - k_rot  # [H, HD]
    kc, ks = np.cos(delta), np.sin(delta)  # [H, HD]
    vcos, vsin = np.cos(v_rot), np.sin(v_rot)
    mk = np.zeros((H, 2 * HD, 2 * HD), np.float32)
    mv = np.zeros((H, 2 * HD, 2 * HD), np.float32)
    ar = np.arange(HD)
    for h in range(H):
        # out[m] = sum_k A[k, m] * z[k];  z = [sr; si]
        mk[h][ar, ar] = kc[h]
        mk[h][HD + ar, ar] = ks[h]
        mk[h][HD + ar, HD + ar] = kc[h]
        mk[h][ar, HD + ar] = -ks[h]
        mv[h][ar, ar] = vcos[h]
        mv[h][HD + ar, ar] = -vsin[h]
        mv[h][ar, HD + ar] = vsin[h]
        mv[h][HD + ar, HD + ar] = vcos[h]

    stepm = np.zeros((P, 1280), np.float32)
    ii = np.arange(1280)[None, :] - 640
    stepm[np.arange(P)[:, None] <= ii] = 1.0

    ffA = np.stack(
        [
            np.concatenate(
                [ff_real[h * HD : (h + 1) * HD, :], -ff_imag[h * HD : (h + 1) * HD, :]],
                axis=0,
            )
            for h in range(H)
        ]
    ).astype(np.float32)  # [H, 128, D]
    ffB = np.stack(
        [
            np.concatenate(
                [ff_imag[h * HD : (h + 1) * HD, :], ff_real[h * HD : (h + 1) * HD, :]],
                axis=0,
            )
            for h in range(H)
        ]
    ).astype(np.float32)

    common = dict(
        xidx=np.ascontiguousarray(x.reshape(SB, 1).astype(np.int32)),
        emb=emb,
        cph=cph,
        sph=sph,
        mk=mk,
        mv=mv,
        stepm=stepm,
        ones=np.ones((P, P), np.float32),
        ffA=ffA,
        ffB=ffB,
    )

    per_core = []
    bias = b_r + b_i
    for c in range(ncores):
        sl = slice(c * Vc, (c + 1) * Vc)
        wr = np.zeros((D, vcp), np.float32)
        wr[:, :Vc] = w_r[:, sl]
        wi = np.zeros((D, vcp), np.float32)
        wi[:, :Vc] = w_i[:, sl]
        w2 = np.zeros((KT, P, vcp), np.float32)
        for kt in range(DT):
            w2[kt] = wr[kt * P : (kt + 1) * P, :]
            w2[DT + kt] = wi[kt * P : (kt + 1) * P, :]
        bb = np.zeros((vcp,), np.float32)
        bb[:Vc] = bias[sl]
        bias2 = np.ascontiguousarray(bb.reshape(vcp // P, P).T)  # [P, NVT]
        per_core.append(dict(w2t=w2, bias2=bias2))
    return common, per_core, (B, S, V, D, H, SB, Vc, vcp)


_NC_CACHE = {}


def kernel(x, emb, q_rot, k_rot, v_rot, ff_real, ff_imag, w_r, b_r, w_i, b_i):
    from concourse.bass_utils import run_bass_kernel_spmd

    common, per_core, meta = host_prep(
        x, emb, q_rot, k_rot, v_rot, ff_real, ff_imag, w_r, b_r, w_i, b_i
    )
    B, S, V, D, H, SB, Vc, vcp = meta

    key = (B, S, V, D, H, vcp)
    if key not in _NC_CACHE:
        _NC_CACHE[key] = build_nc(B, S, V, D, H, vcp)
    nc = _NC_CACHE[key]

    in_maps = [dict(common, **pc) for pc in per_core]
    res = run_bass_kernel_spmd(nc, in_maps, core_ids=list(range(NCORES)))
    chunks = [res.results[c]["outv"][:Vc, :].T for c in range(NCORES)]
    logits = np.concatenate(chunks, axis=1).reshape(B, S, V)
    return np.ascontiguousarray(logits.astype(np.float32))

